# revision 1
# baseline (speedup 1.0000x reference)
"""MGE velocity kernel for 8 Trainium2 NeuronCores.

out[n] = R_sc[n] * sqrt(mge_c * sum_m c_m*exp(-b_m*R2_sc[n]) + bh_c*R2_sc[n]^-1.5)

The reference's 128-node double-exponential quadrature over-resolves the
integral: Q=16 nodes reproduce the fp32 reference to ~2.4e-7 max rel err
(the reference's own fp32 noise floor). So M = Q*K = 256 exp terms/point.

Device strategy (data parallel, 131072 points/core):
  - r2u = x^2+y^2+z^2 on DVE in natural [128,1024] layout
  - duplicate r2u 4x into [128, 4096]: partition p=(32j+g) holds group g's
    4096 points; j in 0..4 selects which m-term this partition computes
  - 64 ACT Exp instructions, each with per-partition scale/bias APs:
    e = exp(scale_p * r2u + bias_p) = c_m * exp(-b_m * R2_sc), fp16 out
  - TensorE matmul vs 0/1 matrix W[128,32] (W[32j+g, g]=1) accumulates all
    256 terms into PSUM fp32 [32, 4096] (sums the j-blocks + all 64 insts)
  - epilogue via Ln/Exp only (same ACT table set): bh = exp(-1.5*ln r2u + k),
    v = exp(0.5*ln(r2u*vc2) - ln scale)
"""

import numpy as np
from numpy.polynomial.legendre import leggauss

N_CORES = 8
H = W = 1024
N = H * W
N_C = N // N_CORES        # 131072 points per core
P = 128
FN = N_C // P             # 1024 natural free dim
G = 32                    # point groups per core
D = 4                     # duplication factor (m-terms per ACT inst)
F = N_C // G              # 4096 dup free dim
QUAD = 8                  # quadrature nodes actually needed
K = 16                    # MGE components
M = QUAD * K              # 256 exp terms
NI = M // D               # 64 ACT instructions
G_CONST = 0.004301
SOFT = 0.0

_BASS_CACHE = {}


def _build_bass():
    if "nc" in _BASS_CACHE:
        return _BASS_CACHE["nc"]
    import concourse.bass as bass
    import concourse.mybir as mybir
    from concourse import bacc
    from concourse.tile import TileContext

    fp32 = mybir.dt.float32
    fp16 = mybir.dt.float16
    AF = mybir.ActivationFunctionType
    OP = mybir.AluOpType

    nc = bacc.Bacc("TRN2")
    xs = nc.dram_tensor("xs", [P, FN], fp32, kind="ExternalInput")
    ys = nc.dram_tensor("ys", [P, FN], fp32, kind="ExternalInput")
    zs = nc.dram_tensor("zs", [P, FN], fp32, kind="ExternalInput")
    w_in = nc.dram_tensor("w_red", [P, G], fp16, kind="ExternalInput")
    sc_in = nc.dram_tensor("scale_sb", [P, NI], fp32, kind="ExternalInput")
    bi_in = nc.dram_tensor("bias_sb", [P, NI], fp32, kind="ExternalInput")
    ep_in = nc.dram_tensor("eplg", [P, 4], fp32, kind="ExternalInput")
    out = nc.dram_tensor("out", [P, FN], fp32, kind="ExternalOutput")

    with TileContext(nc) as tc:
        with (
            tc.tile_pool(name="singles", bufs=1) as singles,
            tc.tile_pool(name="epool", bufs=4) as epool,
            tc.tile_pool(name="psum", bufs=1, space="PSUM") as psum,
        ):
            x_t = singles.tile([P, FN], fp32)
            y_t = singles.tile([P, FN], fp32)
            z_t = singles.tile([P, FN], fp32)
            w_t = singles.tile([P, G], fp16)
            sc_t = singles.tile([P, NI], fp32)
            bi_t = singles.tile([P, NI], fp32)
            ep_t = singles.tile([P, 4], fp32)
            nc.sync.dma_start(x_t[:], xs[:])
            nc.sync.dma_start(y_t[:], ys[:])
            nc.sync.dma_start(z_t[:], zs[:])
            nc.sync.dma_start(w_t[:], w_in[:])
            nc.sync.dma_start(sc_t[:], sc_in[:])
            nc.sync.dma_start(bi_t[:], bi_in[:])
            nc.sync.dma_start(ep_t[:], ep_in[:])

            # r2u = x^2 + y^2 + z^2 (unscaled; 1/scale^2 folded into coeffs)
            # x^2 on otherwise-idle ACT, y^2/z^2/adds on DVE in parallel
            r2 = singles.tile([P, FN], fp32)
            t2 = singles.tile([P, FN], fp32)
            sx = singles.tile([P, FN], fp32)
            nc.scalar.activation(sx[:], x_t[:], AF.Square)
            nc.vector.tensor_tensor(t2[:], y_t[:], y_t[:], OP.mult)
            nc.vector.tensor_tensor(r2[:], z_t[:], z_t[:], OP.mult)
            nc.vector.tensor_tensor(t2[:], t2[:], sx[:], OP.add)
            nc.vector.tensor_tensor(r2[:], r2[:], t2[:], OP.add)

            # duplicate into [128, 4096]: r2d[32j+g, 1024c+t] = r2[g+32c, t]
            r2d = singles.tile([P, F], fp32)
            for j in range(D):
                for c in range(D):
                    nc.sync.dma_start(
                        r2d[G * j : G * (j + 1), FN * c : FN * (c + 1)],
                        r2[G * c : G * (c + 1), :],
                    )

            # BH term early, natural layout — ACT is otherwise idle while the
            # dup DMAs run. bh = exp(-1.5*ln(r2u) + ln(G*10^m_bh*scale^2))
            lnr2n = singles.tile([P, FN], fp32)
            nc.scalar.activation(lnr2n[:], r2[:], AF.Ln)
            bh_n = singles.tile([P, FN], fp32)
            nc.scalar.activation(
                bh_n[:], lnr2n[:], AF.Exp, bias=ep_t[:, 0:1], scale=-1.5
            )

            # main loop: inst i computes terms m = D*i + j on j-block j
            integ = psum.tile([G, F], fp32)
            for i in range(NI):
                e = epool.tile([P, F], fp16, tag="e")
                # first/last e-tile: 4 column-chunk ACTs so ACT starts on a
                # partially-dup'd r2d / PE drains concurrently at the end
                nch = D if i in (0, NI - 1) else 1
                cw = F // nch
                for ch in range(nch):
                    nc.scalar.activation(
                        e[:, cw * ch : cw * (ch + 1)],
                        r2d[:, cw * ch : cw * (ch + 1)],
                        AF.Exp,
                        bias=bi_t[:, i : i + 1], scale=sc_t[:, i : i + 1],
                    )
                for b in range(F // 512):
                    nc.tensor.matmul(
                        integ[:, 512 * b : 512 * (b + 1)],
                        w_t[:],
                        e[:, 512 * b : 512 * (b + 1)],
                        start=(i == 0),
                        stop=(i == NI - 1),
                    )

            # PSUM (already vc2_mge; mge_c folded into bias) -> SBUF in
            # column chunks (nc.any lets idle ACT help DVE), each chunk's
            # reshape DMA overlaps the next chunk's copy
            mge_g = singles.tile([G, F], fp32)
            integ_n = singles.tile([P, FN], fp32)
            for c in range(D):
                nc.any.tensor_copy(
                    mge_g[:, FN * c : FN * (c + 1)],
                    integ[:, FN * c : FN * (c + 1)],
                )
                nc.sync.dma_start(
                    integ_n[G * c : G * (c + 1), :],
                    mge_g[:, FN * c : FN * (c + 1)],
                )
            # epilogue in column halves to overlap DVE/ACT/DMA
            vc2 = singles.tile([P, FN], fp32)
            tv = singles.tile([P, FN], fp32)
            lntv = singles.tile([P, FN], fp32)
            v = singles.tile([P, FN], fp32)
            HF = FN // 2
            for h in range(2):
                s = slice(HF * h, HF * (h + 1))
                nc.vector.tensor_tensor(vc2[:, s], integ_n[:, s], bh_n[:, s], OP.add)
                nc.vector.tensor_tensor(tv[:, s], vc2[:, s], r2[:, s], OP.mult)
                nc.scalar.activation(lntv[:, s], tv[:, s], AF.Ln)
                nc.scalar.activation(
                    v[:, s], lntv[:, s], AF.Exp, bias=ep_t[:, 2:3], scale=0.5
                )
                nc.sync.dma_start(out[:, s], v[:, s])

    nc.compile()
    _BASS_CACHE["nc"] = nc
    return nc


def _host_coeffs(surf, sigma, qobs, M_to_L, inc, m_bh):
    """fp64 host-side reduction of the small parameter vectors to per-term
    (b_m, c_m) plus epilogue constants. Mirrors reference.py's math."""
    surf = surf.astype(np.float64)
    sigma = sigma.astype(np.float64)
    qobs = qobs.astype(np.float64)
    cos_i, sin_i = np.cos(inc), np.sin(inc)
    q_intr = np.sqrt(qobs**2 - cos_i**2) / sin_i
    md = surf * M_to_L * qobs / (q_intr * sigma * np.sqrt(2.0 * np.pi))
    scale = np.quantile(sigma, 0.5)
    sig_sc = sigma / scale
    mds = np.quantile(sig_sc, 0.5)
    mxs = sig_sc.max()
    t_lo = np.arcsinh(np.log(1e-7 * mds) * 2.0 / np.pi)
    t_hi = np.arcsinh(np.log(1000.0 * mxs) * 2.0 / np.pi)
    xl, wl = leggauss(QUAD)
    t = 0.5 * (t_hi - t_lo) * xl + 0.5 * (t_hi + t_lo)
    w = 0.5 * (t_hi - t_lo) * wl
    u = np.exp(np.pi / 2.0 * np.sinh(t))
    du = np.pi / 2.0 * np.cosh(t) * u
    coef = q_intr * md
    inv_s2 = 1.0 / sig_sc**2
    a_j = 0.5 / (1.0 + u)
    b = (a_j[:, None] * inv_s2[None, :]).ravel()          # [M] per R2_sc
    c = (
        (coef[None, :] / ((1.0 + u[:, None]) ** 2
                          * np.sqrt(q_intr[None, :] ** 2 + u[:, None])))
        * (du * w)[:, None]
    ).ravel()                                             # [M]
    assert np.all(c > 0)
    b_eff = b / scale**2                                  # per unscaled r2u
    mge_c = 2.0 * np.pi * G_CONST * scale**2
    c = c * mge_c               # PSUM accumulates vc2_mge directly
    assert c.max() < 6.0e4, "c_m overflows fp16"
    bh_bias = np.log(G_CONST) + m_bh * np.log(10.0) + 2.0 * np.log(scale)
    v_bias = -np.log(scale)
    return b_eff, c, mge_c, bh_bias, v_bias


def kernel(x, y, z, surf, sigma, qobs, M_to_L, inc, m_bh, quad_points):
    from concourse.bass_utils import run_bass_kernel_spmd

    x = np.asarray(x, dtype=np.float32)
    y = np.asarray(y, dtype=np.float32)
    z = np.asarray(z, dtype=np.float32)
    b_eff, c, mge_c, bh_bias, v_bias = _host_coeffs(
        np.asarray(surf), np.asarray(sigma), np.asarray(qobs),
        float(M_to_L), float(inc), float(m_bh),
    )

    # per-partition scale/bias tables: partition p = 32j+g -> term m = D*i+j
    jj = np.arange(P) // G                                # j index per partition
    scale_sb = np.empty((P, NI), np.float32)
    bias_sb = np.empty((P, NI), np.float32)
    for i in range(NI):
        m = D * i + jj
        scale_sb[:, i] = -b_eff[m]
        bias_sb[:, i] = np.log(c[m])
    w_red = np.zeros((P, G), np.float16)
    w_red[np.arange(P), np.arange(P) % G] = 1.0
    eplg = np.zeros((P, 4), np.float32)
    eplg[:, 0] = bh_bias
    eplg[:, 1] = mge_c
    eplg[:, 2] = v_bias

    xf = x.ravel().reshape(N_CORES, P, FN)
    yf = y.ravel().reshape(N_CORES, P, FN)
    zf = z.ravel().reshape(N_CORES, P, FN)
    in_maps = [
        {
            "xs": xf[i], "ys": yf[i], "zs": zf[i],
            "w_red": w_red, "scale_sb": scale_sb, "bias_sb": bias_sb,
            "eplg": eplg,
        }
        for i in range(N_CORES)
    ]
    nc = _build_bass()
    res = run_bass_kernel_spmd(nc, in_maps, core_ids=list(range(N_CORES)))
    outs = [res.results[i]["out"].reshape(-1) for i in range(N_CORES)]
    return np.concatenate(outs).reshape(H, W).astype(np.float32)



# revision 4
# speedup vs baseline: 10.2446x; 10.2446x over previous
"""MGE velocity kernel for 8 Trainium2 NeuronCores.

out[n] = R_sc[n] * sqrt(vc2_mge[n] + vc2_bh[n]),   R2 = x^2+y^2+z^2

Key observation: with these inputs (m_bh = 8 -> 10^8 BH mass) the black-hole
term dominates the MGE integral everywhere the data lives:
    x(r2) := vc2_mge / vc2_bh <= 5.8e-5  over r2 in [r2_min, r2_max].
Since v = R_sc*sqrt(bh)*sqrt(1+x) and sqrt(1+x) = 1 + x/2 + O(x^2), dropping
the MGE term entirely changes v by at most x_max/2 ~ 2.9e-5 relative — far
below the 2e-2 gate (and below the fp32 noise of the reference itself).
Moreover R_sc*sqrt(bh) = sqrt(G*10^m_bh) * r2^{-1/4}  (scale cancels), so

    v = exp(-0.25*ln(r2) + C),   C = 0.5*(ln G + m_bh*ln 10).

The kernel computes x_max at runtime from the actual inputs (exp-sum of the
exact Q=64 quadrature on a log grid over the data's r2 range) and only takes
this fast path when x_max < 1e-3; otherwise it falls back to the full
128-term quadrature kernel (proven baseline, bit-identical code path).

Fast-path device program (per core, 131072 points as [128, 1024]):
  - inputs packed host-side into one dram tensor [128, 3072], chunk-major
    [x|y|z] per 256-col chunk -> 1 input DMA per chunk (DMA count matters:
    HWDGE costs ~630ns per copy, serialized)
  - per chunk: x^2 on DVE, y^2 on Pool(GPSIMD), z^2 on ACT(Square),
    two adds on DVE, Ln on ACT, Exp(scale=-0.25, bias=C) on ACT, DMA out
  - one pre-placed InstLoadActFuncSet(natural_log_exp_and_others) serves
    Square+Ln+Exp from a single table: avoids the compiler's greedy
    per-function table reloads (1.28us each, 5 of them in the baseline)
  - 4 chunks pipeline compute under the ~5.8us of serialized DMA traffic
"""

import numpy as np
from numpy.polynomial.legendre import leggauss

N_CORES = 8
H = W = 1024
N = H * W
P = 128
FN = N // N_CORES // P    # 1024 natural free dim per core
NCH = 4                   # fast-path pipeline chunks
CW = FN // NCH            # 256 columns per chunk
G_CONST = 0.004301
SOFT = 0.0
X_TAYLOR_MAX = 1e-3       # max vc2_mge/vc2_bh for the fast path (err <= x/2)

# ---- slow-path (generic) constants: proven baseline kernel ----
QUAD = 8                  # quadrature nodes for the fallback kernel
K = 16                    # MGE components
M = QUAD * K              # exp terms
G_GRP = 32                # point groups per core (fallback layout)
D = 4                     # duplication factor
F = (N // N_CORES) // G_GRP
NI = M // D               # ACT instructions in fallback main loop

_BASS_CACHE = {}
_ACT_COMBINED_SET = None  # resolved lazily: table containing ln+exp+square


def _combined_act_set_id(nc):
    """Index of the activation-function table that serves ln, exp and
    square together (natural_log_exp_and_others on gen3)."""
    global _ACT_COMBINED_SET
    if _ACT_COMBINED_SET is None:
        import concourse.mybir as mybir
        from concourse.hw_specs import get_activation_tables
        AF = mybir.ActivationFunctionType
        need = {AF.Ln, AF.Exp, AF.Square}
        try:
            tables = list(get_activation_tables(nc.m.arch).values())
        except Exception:
            tables = []
        for idx, funcs in enumerate(tables):
            if need.issubset(funcs):
                _ACT_COMBINED_SET = idx
                break
        else:
            _ACT_COMBINED_SET = -1  # no combined table; let bacc insert loads
    return _ACT_COMBINED_SET


def _build_fast():
    """v = exp(-0.25*ln(x^2+y^2+z^2) + C) pipeline."""
    if "fast" in _BASS_CACHE:
        return _BASS_CACHE["fast"]
    import concourse.mybir as mybir
    from concourse import bacc
    from concourse.tile import TileContext

    fp32 = mybir.dt.float32
    AF = mybir.ActivationFunctionType
    OP = mybir.AluOpType

    nc = bacc.Bacc("TRN2")
    xyz_in = nc.dram_tensor("xyzp", [P, 3 * FN], fp32, kind="ExternalInput")
    cst_in = nc.dram_tensor("cst", [P, 8], fp32, kind="ExternalInput")
    out = nc.dram_tensor("out", [P, FN], fp32, kind="ExternalOutput")

    with TileContext(nc) as tc:
        with tc.tile_pool(name="singles", bufs=1) as sg:
            set_id = _combined_act_set_id(nc)
            if set_id >= 0:
                nc.scalar.add_instruction(mybir.InstLoadActFuncSet(
                    name=nc.get_next_instruction_name(),
                    act_func_set_id=set_id, ins=[], outs=[]))

            xyz_t = sg.tile([P, 3 * FN], fp32)
            cst_t = sg.tile([P, 8], fp32)
            sx = sg.tile([P, FN], fp32)
            sy = sg.tile([P, FN], fp32)
            sz = sg.tile([P, FN], fp32)
            t1 = sg.tile([P, FN], fp32)
            r2 = sg.tile([P, FN], fp32)
            lnr2 = sg.tile([P, FN], fp32)
            v = sg.tile([P, FN], fp32)

            nc.sync.dma_start(cst_t[:], cst_in[:])
            for ch in range(NCH):
                s3 = slice(3 * CW * ch, 3 * CW * (ch + 1))
                nc.sync.dma_start(xyz_t[:, s3], xyz_in[:, s3])

            for ch in range(NCH):
                o = 3 * CW * ch
                s = slice(CW * ch, CW * (ch + 1))
                xs = xyz_t[:, o : o + CW]
                ys = xyz_t[:, o + CW : o + 2 * CW]
                zs = xyz_t[:, o + 2 * CW : o + 3 * CW]
                nc.vector.tensor_tensor(sx[:, s], xs, xs, OP.mult)
                nc.gpsimd.tensor_tensor(sy[:, s], ys, ys, OP.mult)
                nc.scalar.activation(sz[:, s], zs, AF.Square)
                nc.vector.tensor_tensor(t1[:, s], sx[:, s], sy[:, s], OP.add)
                nc.vector.tensor_tensor(r2[:, s], t1[:, s], sz[:, s], OP.add)
                nc.scalar.activation(lnr2[:, s], r2[:, s], AF.Ln)
                nc.scalar.activation(
                    v[:, s], lnr2[:, s], AF.Exp, bias=cst_t[:, 0:1], scale=-0.25
                )
                nc.sync.dma_start(out[:, s], v[:, s])

    nc.compile()
    _BASS_CACHE["fast"] = nc
    return nc


def _build_full():
    """Fallback: full 128-term quadrature kernel (baseline, unchanged)."""
    if "full" in _BASS_CACHE:
        return _BASS_CACHE["full"]
    import concourse.mybir as mybir
    from concourse import bacc
    from concourse.tile import TileContext

    fp32 = mybir.dt.float32
    fp16 = mybir.dt.float16
    AF = mybir.ActivationFunctionType
    OP = mybir.AluOpType

    nc = bacc.Bacc("TRN2")
    xs = nc.dram_tensor("xs", [P, FN], fp32, kind="ExternalInput")
    ys = nc.dram_tensor("ys", [P, FN], fp32, kind="ExternalInput")
    zs = nc.dram_tensor("zs", [P, FN], fp32, kind="ExternalInput")
    w_in = nc.dram_tensor("w_red", [P, G_GRP], fp16, kind="ExternalInput")
    sc_in = nc.dram_tensor("scale_sb", [P, NI], fp32, kind="ExternalInput")
    bi_in = nc.dram_tensor("bias_sb", [P, NI], fp32, kind="ExternalInput")
    ep_in = nc.dram_tensor("eplg", [P, 4], fp32, kind="ExternalInput")
    out = nc.dram_tensor("out", [P, FN], fp32, kind="ExternalOutput")

    with TileContext(nc) as tc:
        with (
            tc.tile_pool(name="singles", bufs=1) as singles,
            tc.tile_pool(name="epool", bufs=4) as epool,
            tc.tile_pool(name="psum", bufs=1, space="PSUM") as psum,
        ):
            x_t = singles.tile([P, FN], fp32)
            y_t = singles.tile([P, FN], fp32)
            z_t = singles.tile([P, FN], fp32)
            w_t = singles.tile([P, G_GRP], fp16)
            sc_t = singles.tile([P, NI], fp32)
            bi_t = singles.tile([P, NI], fp32)
            ep_t = singles.tile([P, 4], fp32)
            nc.sync.dma_start(x_t[:], xs[:])
            nc.sync.dma_start(y_t[:], ys[:])
            nc.sync.dma_start(z_t[:], zs[:])
            nc.sync.dma_start(w_t[:], w_in[:])
            nc.sync.dma_start(sc_t[:], sc_in[:])
            nc.sync.dma_start(bi_t[:], bi_in[:])
            nc.sync.dma_start(ep_t[:], ep_in[:])

            r2 = singles.tile([P, FN], fp32)
            t2 = singles.tile([P, FN], fp32)
            sx = singles.tile([P, FN], fp32)
            nc.scalar.activation(sx[:], x_t[:], AF.Square)
            nc.vector.tensor_tensor(t2[:], y_t[:], y_t[:], OP.mult)
            nc.vector.tensor_tensor(r2[:], z_t[:], z_t[:], OP.mult)
            nc.vector.tensor_tensor(t2[:], t2[:], sx[:], OP.add)
            nc.vector.tensor_tensor(r2[:], r2[:], t2[:], OP.add)

            r2d = singles.tile([P, F], fp32)
            for j in range(D):
                for c in range(D):
                    nc.sync.dma_start(
                        r2d[G_GRP * j : G_GRP * (j + 1), FN * c : FN * (c + 1)],
                        r2[G_GRP * c : G_GRP * (c + 1), :],
                    )

            lnr2n = singles.tile([P, FN], fp32)
            nc.scalar.activation(lnr2n[:], r2[:], AF.Ln)
            bh_n = singles.tile([P, FN], fp32)
            nc.scalar.activation(
                bh_n[:], lnr2n[:], AF.Exp, bias=ep_t[:, 0:1], scale=-1.5
            )

            integ = psum.tile([G_GRP, F], fp32)
            for i in range(NI):
                e = epool.tile([P, F], fp16, tag="e")
                nch = D if i in (0, NI - 1) else 1
                cw = F // nch
                for ch in range(nch):
                    nc.scalar.activation(
                        e[:, cw * ch : cw * (ch + 1)],
                        r2d[:, cw * ch : cw * (ch + 1)],
                        AF.Exp,
                        bias=bi_t[:, i : i + 1], scale=sc_t[:, i : i + 1],
                    )
                for b in range(F // 512):
                    nc.tensor.matmul(
                        integ[:, 512 * b : 512 * (b + 1)],
                        w_t[:],
                        e[:, 512 * b : 512 * (b + 1)],
                        start=(i == 0),
                        stop=(i == NI - 1),
                    )

            mge_g = singles.tile([G_GRP, F], fp32)
            integ_n = singles.tile([P, FN], fp32)
            for c in range(D):
                nc.any.tensor_copy(
                    mge_g[:, FN * c : FN * (c + 1)],
                    integ[:, FN * c : FN * (c + 1)],
                )
                nc.sync.dma_start(
                    integ_n[G_GRP * c : G_GRP * (c + 1), :],
                    mge_g[:, FN * c : FN * (c + 1)],
                )
            vc2 = singles.tile([P, FN], fp32)
            tv = singles.tile([P, FN], fp32)
            lntv = singles.tile([P, FN], fp32)
            v = singles.tile([P, FN], fp32)
            HF = FN // 2
            for h in range(2):
                s = slice(HF * h, HF * (h + 1))
                nc.vector.tensor_tensor(vc2[:, s], integ_n[:, s], bh_n[:, s], OP.add)
                nc.vector.tensor_tensor(tv[:, s], vc2[:, s], r2[:, s], OP.mult)
                nc.scalar.activation(lntv[:, s], tv[:, s], AF.Ln)
                nc.scalar.activation(
                    v[:, s], lntv[:, s], AF.Exp, bias=ep_t[:, 2:3], scale=0.5
                )
                nc.sync.dma_start(out[:, s], v[:, s])

    nc.compile()
    _BASS_CACHE["full"] = nc
    return nc


def _quad_terms(surf, sigma, qobs, M_to_L, inc, quad):
    """fp64 (b_m, c_m) exp-sum terms of vc2_mge in UNSCALED r2, with the
    2*pi*G*scale^2 prefactor folded into c. Mirrors reference.py's math."""
    surf = surf.astype(np.float64)
    sigma = sigma.astype(np.float64)
    qobs = qobs.astype(np.float64)
    cos_i, sin_i = np.cos(inc), np.sin(inc)
    q_intr = np.sqrt(qobs**2 - cos_i**2) / sin_i
    md = surf * M_to_L * qobs / (q_intr * sigma * np.sqrt(2.0 * np.pi))
    scale = np.quantile(sigma, 0.5)
    sig_sc = sigma / scale
    mds = np.quantile(sig_sc, 0.5)
    mxs = sig_sc.max()
    t_lo = np.arcsinh(np.log(1e-7 * mds) * 2.0 / np.pi)
    t_hi = np.arcsinh(np.log(1000.0 * mxs) * 2.0 / np.pi)
    xl, wl = leggauss(quad)
    t = 0.5 * (t_hi - t_lo) * xl + 0.5 * (t_hi + t_lo)
    w = 0.5 * (t_hi - t_lo) * wl
    u = np.exp(np.pi / 2.0 * np.sinh(t))
    du = np.pi / 2.0 * np.cosh(t) * u
    coef = q_intr * md
    inv_s2 = 1.0 / sig_sc**2
    b = ((0.5 / (1.0 + u))[:, None] * inv_s2[None, :]).ravel() / scale**2
    c = (
        (coef[None, :] / ((1.0 + u[:, None]) ** 2
                          * np.sqrt(q_intr[None, :] ** 2 + u[:, None])))
        * (du * w)[:, None]
    ).ravel()
    mge_c = 2.0 * np.pi * G_CONST * scale**2
    return b, c * mge_c, scale


def _x_max(surf, sigma, qobs, M_to_L, inc, m_bh, r2_min, r2_max):
    """max over the data's r2 range of vc2_mge/vc2_bh (exact Q=64 sum)."""
    b, c, scale = _quad_terms(surf, sigma, qobs, M_to_L, inc, 64)
    lo = max(float(r2_min) * 0.5, 1e-30)
    hi = float(r2_max) * 2.0
    grid = np.geomspace(lo, hi, 512)
    f = np.exp(-np.outer(grid, b)) @ c
    bh_coef = G_CONST * 10.0 ** m_bh * scale**2   # vc2_bh = bh_coef*r2^-1.5
    if not np.isfinite(bh_coef) or bh_coef <= 0.0:
        return np.inf
    bh = bh_coef * grid ** -1.5
    return float(np.max(f / bh))


def _host_coeffs_full(surf, sigma, qobs, M_to_L, inc, m_bh):
    """Host-side reduction for the fallback kernel (as in the baseline)."""
    b, c, scale = _quad_terms(surf, sigma, qobs, M_to_L, inc, QUAD)
    b_eff = b
    mge_c = 2.0 * np.pi * G_CONST * scale**2
    assert np.all(c > 0)
    assert c.max() < 6.0e4, "c_m overflows fp16"
    bh_bias = np.log(G_CONST) + m_bh * np.log(10.0) + 2.0 * np.log(scale)
    v_bias = -np.log(scale)
    return b_eff, c, mge_c, bh_bias, v_bias


def _run_fast(x, y, z, m_bh):
    from concourse.bass_utils import run_bass_kernel_spmd

    # pack [x|y|z] chunk-major per core: (cores, P, NCH, 3, CW) -> (P, 3*FN)
    xyz = np.stack(
        [
            x.ravel().reshape(N_CORES, P, NCH, CW),
            y.ravel().reshape(N_CORES, P, NCH, CW),
            z.ravel().reshape(N_CORES, P, NCH, CW),
        ],
        axis=3,
    ).reshape(N_CORES, P, 3 * FN)
    cst = np.zeros((P, 8), np.float32)
    cst[:, 0] = 0.5 * (np.log(G_CONST) + float(m_bh) * np.log(10.0))
    in_maps = [{"xyzp": xyz[i], "cst": cst} for i in range(N_CORES)]
    nc = _build_fast()
    res = run_bass_kernel_spmd(nc, in_maps, core_ids=list(range(N_CORES)))
    outs = [res.results[i]["out"].reshape(-1) for i in range(N_CORES)]
    return np.concatenate(outs).reshape(H, W).astype(np.float32)


def _run_full(x, y, z, surf, sigma, qobs, M_to_L, inc, m_bh):
    from concourse.bass_utils import run_bass_kernel_spmd

    b_eff, c, mge_c, bh_bias, v_bias = _host_coeffs_full(
        np.asarray(surf), np.asarray(sigma), np.asarray(qobs),
        float(M_to_L), float(inc), float(m_bh),
    )
    jj = np.arange(P) // G_GRP
    scale_sb = np.empty((P, NI), np.float32)
    bias_sb = np.empty((P, NI), np.float32)
    for i in range(NI):
        m = D * i + jj
        scale_sb[:, i] = -b_eff[m]
        bias_sb[:, i] = np.log(c[m])
    w_red = np.zeros((P, G_GRP), np.float16)
    w_red[np.arange(P), np.arange(P) % G_GRP] = 1.0
    eplg = np.zeros((P, 4), np.float32)
    eplg[:, 0] = bh_bias
    eplg[:, 1] = mge_c
    eplg[:, 2] = v_bias

    xf = x.ravel().reshape(N_CORES, P, FN)
    yf = y.ravel().reshape(N_CORES, P, FN)
    zf = z.ravel().reshape(N_CORES, P, FN)
    in_maps = [
        {
            "xs": xf[i], "ys": yf[i], "zs": zf[i],
            "w_red": w_red, "scale_sb": scale_sb, "bias_sb": bias_sb,
            "eplg": eplg,
        }
        for i in range(N_CORES)
    ]
    nc = _build_full()
    res = run_bass_kernel_spmd(nc, in_maps, core_ids=list(range(N_CORES)))
    outs = [res.results[i]["out"].reshape(-1) for i in range(N_CORES)]
    return np.concatenate(outs).reshape(H, W).astype(np.float32)


def kernel(x, y, z, surf, sigma, qobs, M_to_L, inc, m_bh, quad_points):
    x = np.asarray(x, dtype=np.float32)
    y = np.asarray(y, dtype=np.float32)
    z = np.asarray(z, dtype=np.float32)

    r2 = (x.astype(np.float64) ** 2 + y.astype(np.float64) ** 2
          + z.astype(np.float64) ** 2)
    r2_min, r2_max = float(r2.min()), float(r2.max())
    try:
        xm = _x_max(np.asarray(surf), np.asarray(sigma), np.asarray(qobs),
                    float(M_to_L), float(inc), float(m_bh), r2_min, r2_max)
    except Exception:
        xm = np.inf

    if xm < X_TAYLOR_MAX and r2_min > 0.0:
        return _run_fast(x, y, z, float(m_bh))
    return _run_full(x, y, z, surf, sigma, qobs, M_to_L, inc, m_bh)


# revision 5
# speedup vs baseline: 10.4435x; 1.0194x over previous
"""MGE velocity kernel for 8 Trainium2 NeuronCores.

out[n] = R_sc[n] * sqrt(vc2_mge[n] + vc2_bh[n]),   R2 = x^2+y^2+z^2

Key observation: with these inputs (m_bh = 8 -> 10^8 BH mass) the black-hole
term dominates the MGE integral everywhere the data lives:
    x(r2) := vc2_mge / vc2_bh <= 5.8e-5  over r2 in [r2_min, r2_max].
Since v = R_sc*sqrt(bh)*sqrt(1+x) and sqrt(1+x) = 1 + x/2 + O(x^2), dropping
the MGE term entirely changes v by at most x_max/2 ~ 2.9e-5 relative — far
below the 2e-2 gate (and below the fp32 noise of the reference itself).
Moreover R_sc*sqrt(bh) = sqrt(G*10^m_bh) * r2^{-1/4}  (scale cancels), so

    v = exp(-0.25*ln(r2) + C),   C = 0.5*(ln G + m_bh*ln 10).

The kernel computes x_max at runtime from the actual inputs (exp-sum of the
exact Q=64 quadrature on a log grid over the data's r2 range) and only takes
this fast path when x_max < 1e-3; otherwise it falls back to the full
128-term quadrature kernel (proven baseline, bit-identical code path).

Fast-path device program (per core, 131072 points as [128, 1024]):
  - inputs packed host-side into one dram tensor [128, 3072], chunk-major
    [x|y|z] per 256-col chunk -> 1 input DMA per chunk (DMA count matters:
    HWDGE costs ~630ns per copy, serialized)
  - per chunk: x^2 on DVE, y^2 on Pool(GPSIMD), z^2 on ACT(Square),
    two adds on DVE, Ln on ACT, Exp(scale=-0.25, bias=C) on ACT, DMA out
  - one pre-placed InstLoadActFuncSet(natural_log_exp_and_others) serves
    Square+Ln+Exp from a single table: avoids the compiler's greedy
    per-function table reloads (1.28us each, 5 of them in the baseline)
  - 4 chunks pipeline compute under the ~5.8us of serialized DMA traffic
"""

import numpy as np
from numpy.polynomial.legendre import leggauss

N_CORES = 8
H = W = 1024
N = H * W
P = 128
FN = N // N_CORES // P    # 1024 natural free dim per core
NCH = 4                   # fast-path pipeline chunks
CW = FN // NCH            # 256 columns per chunk
G_CONST = 0.004301
SOFT = 0.0
X_TAYLOR_MAX = 1e-3       # max vc2_mge/vc2_bh for the fast path (err <= x/2)

# ---- slow-path (generic) constants: proven baseline kernel ----
QUAD = 8                  # quadrature nodes for the fallback kernel
K = 16                    # MGE components
M = QUAD * K              # exp terms
G_GRP = 32                # point groups per core (fallback layout)
D = 4                     # duplication factor
F = (N // N_CORES) // G_GRP
NI = M // D               # ACT instructions in fallback main loop

_BASS_CACHE = {}
_ACT_COMBINED_SET = None  # resolved lazily: table containing ln+exp+square


def _combined_act_set_id(nc):
    """Index of the activation-function table that serves ln, exp and
    square together (natural_log_exp_and_others on gen3)."""
    global _ACT_COMBINED_SET
    if _ACT_COMBINED_SET is None:
        import concourse.mybir as mybir
        from concourse.hw_specs import get_activation_tables
        AF = mybir.ActivationFunctionType
        need = {AF.Ln, AF.Exp, AF.Square}
        try:
            tables = list(get_activation_tables(nc.m.arch).values())
        except Exception:
            tables = []
        for idx, funcs in enumerate(tables):
            if need.issubset(funcs):
                _ACT_COMBINED_SET = idx
                break
        else:
            _ACT_COMBINED_SET = -1  # no combined table; let bacc insert loads
    return _ACT_COMBINED_SET


def _build_fast():
    """v = exp(-0.25*ln(x^2+y^2+z^2) + C) pipeline."""
    if "fast" in _BASS_CACHE:
        return _BASS_CACHE["fast"]
    import concourse.mybir as mybir
    from concourse import bacc
    from concourse.tile import TileContext

    fp32 = mybir.dt.float32
    AF = mybir.ActivationFunctionType
    OP = mybir.AluOpType

    nc = bacc.Bacc("TRN2")
    xyz_in = nc.dram_tensor("xyzp", [P, 3 * FN], fp32, kind="ExternalInput")
    cst_in = nc.dram_tensor("cst", [P, 8], fp32, kind="ExternalInput")
    out = nc.dram_tensor("out", [P, FN], fp32, kind="ExternalOutput")

    with TileContext(nc) as tc:
        with tc.tile_pool(name="singles", bufs=1) as sg:
            set_id = _combined_act_set_id(nc)
            if set_id >= 0:
                nc.scalar.add_instruction(mybir.InstLoadActFuncSet(
                    name=nc.get_next_instruction_name(),
                    act_func_set_id=set_id, ins=[], outs=[]))

            xyz_t = sg.tile([P, 3 * FN], fp32)
            cst_t = sg.tile([P, 8], fp32)
            sx = sg.tile([P, FN], fp32)
            sy = sg.tile([P, FN], fp32)
            sz = sg.tile([P, FN], fp32)
            t1 = sg.tile([P, FN], fp32)
            r2 = sg.tile([P, FN], fp32)
            lnr2 = sg.tile([P, FN], fp32)
            v = sg.tile([P, FN], fp32)

            for ch in range(NCH):
                s3 = slice(3 * CW * ch, 3 * CW * (ch + 1))
                nc.sync.dma_start(xyz_t[:, s3], xyz_in[:, s3])
            # cst is only read by chunk 0's final Exp — issue it after the
            # input chunks so it doesn't hold the first HWDGE slot
            nc.sync.dma_start(cst_t[:], cst_in[:])

            for ch in range(NCH):
                o = 3 * CW * ch
                s = slice(CW * ch, CW * (ch + 1))
                xs = xyz_t[:, o : o + CW]
                ys = xyz_t[:, o + CW : o + 2 * CW]
                zs = xyz_t[:, o + 2 * CW : o + 3 * CW]
                nc.vector.tensor_tensor(sx[:, s], xs, xs, OP.mult)
                nc.gpsimd.tensor_tensor(sy[:, s], ys, ys, OP.mult)
                # chunk 0: z^2 on ACT (DVE/Pool busy, ACT idle after the table
                # load); later chunks: z^2 on DVE so ACT only runs Ln+Exp and
                # never backlogs behind the Square
                if ch == 0:
                    nc.scalar.activation(sz[:, s], zs, AF.Square)
                else:
                    nc.vector.tensor_tensor(sz[:, s], zs, zs, OP.mult)
                nc.vector.tensor_tensor(t1[:, s], sx[:, s], sy[:, s], OP.add)
                nc.vector.tensor_tensor(r2[:, s], t1[:, s], sz[:, s], OP.add)
                # last chunk: halve the Ln/Exp/out so the final out DMA starts
                # as soon as the first half's v is ready (shorter tail)
                nhalf = 2 if ch == NCH - 1 else 1
                hw_ = CW // nhalf
                for hh in range(nhalf):
                    sh = slice(CW * ch + hw_ * hh, CW * ch + hw_ * (hh + 1))
                    nc.scalar.activation(lnr2[:, sh], r2[:, sh], AF.Ln)
                    nc.scalar.activation(
                        v[:, sh], lnr2[:, sh], AF.Exp,
                        bias=cst_t[:, 0:1], scale=-0.25,
                    )
                    nc.sync.dma_start(out[:, sh], v[:, sh])

    nc.compile()
    _BASS_CACHE["fast"] = nc
    return nc


def _build_full():
    """Fallback: full 128-term quadrature kernel (baseline, unchanged)."""
    if "full" in _BASS_CACHE:
        return _BASS_CACHE["full"]
    import concourse.mybir as mybir
    from concourse import bacc
    from concourse.tile import TileContext

    fp32 = mybir.dt.float32
    fp16 = mybir.dt.float16
    AF = mybir.ActivationFunctionType
    OP = mybir.AluOpType

    nc = bacc.Bacc("TRN2")
    xs = nc.dram_tensor("xs", [P, FN], fp32, kind="ExternalInput")
    ys = nc.dram_tensor("ys", [P, FN], fp32, kind="ExternalInput")
    zs = nc.dram_tensor("zs", [P, FN], fp32, kind="ExternalInput")
    w_in = nc.dram_tensor("w_red", [P, G_GRP], fp16, kind="ExternalInput")
    sc_in = nc.dram_tensor("scale_sb", [P, NI], fp32, kind="ExternalInput")
    bi_in = nc.dram_tensor("bias_sb", [P, NI], fp32, kind="ExternalInput")
    ep_in = nc.dram_tensor("eplg", [P, 4], fp32, kind="ExternalInput")
    out = nc.dram_tensor("out", [P, FN], fp32, kind="ExternalOutput")

    with TileContext(nc) as tc:
        with (
            tc.tile_pool(name="singles", bufs=1) as singles,
            tc.tile_pool(name="epool", bufs=4) as epool,
            tc.tile_pool(name="psum", bufs=1, space="PSUM") as psum,
        ):
            x_t = singles.tile([P, FN], fp32)
            y_t = singles.tile([P, FN], fp32)
            z_t = singles.tile([P, FN], fp32)
            w_t = singles.tile([P, G_GRP], fp16)
            sc_t = singles.tile([P, NI], fp32)
            bi_t = singles.tile([P, NI], fp32)
            ep_t = singles.tile([P, 4], fp32)
            nc.sync.dma_start(x_t[:], xs[:])
            nc.sync.dma_start(y_t[:], ys[:])
            nc.sync.dma_start(z_t[:], zs[:])
            nc.sync.dma_start(w_t[:], w_in[:])
            nc.sync.dma_start(sc_t[:], sc_in[:])
            nc.sync.dma_start(bi_t[:], bi_in[:])
            nc.sync.dma_start(ep_t[:], ep_in[:])

            r2 = singles.tile([P, FN], fp32)
            t2 = singles.tile([P, FN], fp32)
            sx = singles.tile([P, FN], fp32)
            nc.scalar.activation(sx[:], x_t[:], AF.Square)
            nc.vector.tensor_tensor(t2[:], y_t[:], y_t[:], OP.mult)
            nc.vector.tensor_tensor(r2[:], z_t[:], z_t[:], OP.mult)
            nc.vector.tensor_tensor(t2[:], t2[:], sx[:], OP.add)
            nc.vector.tensor_tensor(r2[:], r2[:], t2[:], OP.add)

            r2d = singles.tile([P, F], fp32)
            for j in range(D):
                for c in range(D):
                    nc.sync.dma_start(
                        r2d[G_GRP * j : G_GRP * (j + 1), FN * c : FN * (c + 1)],
                        r2[G_GRP * c : G_GRP * (c + 1), :],
                    )

            lnr2n = singles.tile([P, FN], fp32)
            nc.scalar.activation(lnr2n[:], r2[:], AF.Ln)
            bh_n = singles.tile([P, FN], fp32)
            nc.scalar.activation(
                bh_n[:], lnr2n[:], AF.Exp, bias=ep_t[:, 0:1], scale=-1.5
            )

            integ = psum.tile([G_GRP, F], fp32)
            for i in range(NI):
                e = epool.tile([P, F], fp16, tag="e")
                nch = D if i in (0, NI - 1) else 1
                cw = F // nch
                for ch in range(nch):
                    nc.scalar.activation(
                        e[:, cw * ch : cw * (ch + 1)],
                        r2d[:, cw * ch : cw * (ch + 1)],
                        AF.Exp,
                        bias=bi_t[:, i : i + 1], scale=sc_t[:, i : i + 1],
                    )
                for b in range(F // 512):
                    nc.tensor.matmul(
                        integ[:, 512 * b : 512 * (b + 1)],
                        w_t[:],
                        e[:, 512 * b : 512 * (b + 1)],
                        start=(i == 0),
                        stop=(i == NI - 1),
                    )

            mge_g = singles.tile([G_GRP, F], fp32)
            integ_n = singles.tile([P, FN], fp32)
            for c in range(D):
                nc.any.tensor_copy(
                    mge_g[:, FN * c : FN * (c + 1)],
                    integ[:, FN * c : FN * (c + 1)],
                )
                nc.sync.dma_start(
                    integ_n[G_GRP * c : G_GRP * (c + 1), :],
                    mge_g[:, FN * c : FN * (c + 1)],
                )
            vc2 = singles.tile([P, FN], fp32)
            tv = singles.tile([P, FN], fp32)
            lntv = singles.tile([P, FN], fp32)
            v = singles.tile([P, FN], fp32)
            HF = FN // 2
            for h in range(2):
                s = slice(HF * h, HF * (h + 1))
                nc.vector.tensor_tensor(vc2[:, s], integ_n[:, s], bh_n[:, s], OP.add)
                nc.vector.tensor_tensor(tv[:, s], vc2[:, s], r2[:, s], OP.mult)
                nc.scalar.activation(lntv[:, s], tv[:, s], AF.Ln)
                nc.scalar.activation(
                    v[:, s], lntv[:, s], AF.Exp, bias=ep_t[:, 2:3], scale=0.5
                )
                nc.sync.dma_start(out[:, s], v[:, s])

    nc.compile()
    _BASS_CACHE["full"] = nc
    return nc


def _quad_terms(surf, sigma, qobs, M_to_L, inc, quad):
    """fp64 (b_m, c_m) exp-sum terms of vc2_mge in UNSCALED r2, with the
    2*pi*G*scale^2 prefactor folded into c. Mirrors reference.py's math."""
    surf = surf.astype(np.float64)
    sigma = sigma.astype(np.float64)
    qobs = qobs.astype(np.float64)
    cos_i, sin_i = np.cos(inc), np.sin(inc)
    q_intr = np.sqrt(qobs**2 - cos_i**2) / sin_i
    md = surf * M_to_L * qobs / (q_intr * sigma * np.sqrt(2.0 * np.pi))
    scale = np.quantile(sigma, 0.5)
    sig_sc = sigma / scale
    mds = np.quantile(sig_sc, 0.5)
    mxs = sig_sc.max()
    t_lo = np.arcsinh(np.log(1e-7 * mds) * 2.0 / np.pi)
    t_hi = np.arcsinh(np.log(1000.0 * mxs) * 2.0 / np.pi)
    xl, wl = leggauss(quad)
    t = 0.5 * (t_hi - t_lo) * xl + 0.5 * (t_hi + t_lo)
    w = 0.5 * (t_hi - t_lo) * wl
    u = np.exp(np.pi / 2.0 * np.sinh(t))
    du = np.pi / 2.0 * np.cosh(t) * u
    coef = q_intr * md
    inv_s2 = 1.0 / sig_sc**2
    b = ((0.5 / (1.0 + u))[:, None] * inv_s2[None, :]).ravel() / scale**2
    c = (
        (coef[None, :] / ((1.0 + u[:, None]) ** 2
                          * np.sqrt(q_intr[None, :] ** 2 + u[:, None])))
        * (du * w)[:, None]
    ).ravel()
    mge_c = 2.0 * np.pi * G_CONST * scale**2
    return b, c * mge_c, scale


def _x_max(surf, sigma, qobs, M_to_L, inc, m_bh, r2_min, r2_max):
    """max over the data's r2 range of vc2_mge/vc2_bh (exact Q=64 sum)."""
    b, c, scale = _quad_terms(surf, sigma, qobs, M_to_L, inc, 64)
    lo = max(float(r2_min) * 0.5, 1e-30)
    hi = float(r2_max) * 2.0
    grid = np.geomspace(lo, hi, 512)
    f = np.exp(-np.outer(grid, b)) @ c
    bh_coef = G_CONST * 10.0 ** m_bh * scale**2   # vc2_bh = bh_coef*r2^-1.5
    if not np.isfinite(bh_coef) or bh_coef <= 0.0:
        return np.inf
    bh = bh_coef * grid ** -1.5
    return float(np.max(f / bh))


def _host_coeffs_full(surf, sigma, qobs, M_to_L, inc, m_bh):
    """Host-side reduction for the fallback kernel (as in the baseline)."""
    b, c, scale = _quad_terms(surf, sigma, qobs, M_to_L, inc, QUAD)
    b_eff = b
    mge_c = 2.0 * np.pi * G_CONST * scale**2
    assert np.all(c > 0)
    assert c.max() < 6.0e4, "c_m overflows fp16"
    bh_bias = np.log(G_CONST) + m_bh * np.log(10.0) + 2.0 * np.log(scale)
    v_bias = -np.log(scale)
    return b_eff, c, mge_c, bh_bias, v_bias


def _run_fast(x, y, z, m_bh):
    from concourse.bass_utils import run_bass_kernel_spmd

    # pack [x|y|z] chunk-major per core: (cores, P, NCH, 3, CW) -> (P, 3*FN)
    xyz = np.stack(
        [
            x.ravel().reshape(N_CORES, P, NCH, CW),
            y.ravel().reshape(N_CORES, P, NCH, CW),
            z.ravel().reshape(N_CORES, P, NCH, CW),
        ],
        axis=3,
    ).reshape(N_CORES, P, 3 * FN)
    cst = np.zeros((P, 8), np.float32)
    cst[:, 0] = 0.5 * (np.log(G_CONST) + float(m_bh) * np.log(10.0))
    in_maps = [{"xyzp": xyz[i], "cst": cst} for i in range(N_CORES)]
    nc = _build_fast()
    res = run_bass_kernel_spmd(nc, in_maps, core_ids=list(range(N_CORES)))
    outs = [res.results[i]["out"].reshape(-1) for i in range(N_CORES)]
    return np.concatenate(outs).reshape(H, W).astype(np.float32)


def _run_full(x, y, z, surf, sigma, qobs, M_to_L, inc, m_bh):
    from concourse.bass_utils import run_bass_kernel_spmd

    b_eff, c, mge_c, bh_bias, v_bias = _host_coeffs_full(
        np.asarray(surf), np.asarray(sigma), np.asarray(qobs),
        float(M_to_L), float(inc), float(m_bh),
    )
    jj = np.arange(P) // G_GRP
    scale_sb = np.empty((P, NI), np.float32)
    bias_sb = np.empty((P, NI), np.float32)
    for i in range(NI):
        m = D * i + jj
        scale_sb[:, i] = -b_eff[m]
        bias_sb[:, i] = np.log(c[m])
    w_red = np.zeros((P, G_GRP), np.float16)
    w_red[np.arange(P), np.arange(P) % G_GRP] = 1.0
    eplg = np.zeros((P, 4), np.float32)
    eplg[:, 0] = bh_bias
    eplg[:, 1] = mge_c
    eplg[:, 2] = v_bias

    xf = x.ravel().reshape(N_CORES, P, FN)
    yf = y.ravel().reshape(N_CORES, P, FN)
    zf = z.ravel().reshape(N_CORES, P, FN)
    in_maps = [
        {
            "xs": xf[i], "ys": yf[i], "zs": zf[i],
            "w_red": w_red, "scale_sb": scale_sb, "bias_sb": bias_sb,
            "eplg": eplg,
        }
        for i in range(N_CORES)
    ]
    nc = _build_full()
    res = run_bass_kernel_spmd(nc, in_maps, core_ids=list(range(N_CORES)))
    outs = [res.results[i]["out"].reshape(-1) for i in range(N_CORES)]
    return np.concatenate(outs).reshape(H, W).astype(np.float32)


def kernel(x, y, z, surf, sigma, qobs, M_to_L, inc, m_bh, quad_points):
    x = np.asarray(x, dtype=np.float32)
    y = np.asarray(y, dtype=np.float32)
    z = np.asarray(z, dtype=np.float32)

    r2 = (x.astype(np.float64) ** 2 + y.astype(np.float64) ** 2
          + z.astype(np.float64) ** 2)
    r2_min, r2_max = float(r2.min()), float(r2.max())
    try:
        xm = _x_max(np.asarray(surf), np.asarray(sigma), np.asarray(qobs),
                    float(M_to_L), float(inc), float(m_bh), r2_min, r2_max)
    except Exception:
        xm = np.inf

    if xm < X_TAYLOR_MAX and r2_min > 0.0:
        return _run_fast(x, y, z, float(m_bh))
    return _run_full(x, y, z, surf, sigma, qobs, M_to_L, inc, m_bh)
